# revision 41
# baseline (speedup 1.0000x reference)
"""Causal self-attention (B=2, N=2048, D=768, H=12) on 8 Trainium2 NeuronCores.

Sharding: data-parallel over batch (2) x tensor-parallel over head groups (4),
3 heads per core. Each core computes, for its (batch, head-group):
  GEMM1: kT/qT (transposed) and v (natural) projections from xT,
  scores^T = k @ q^T per head, exp on ScalarE (fp16 out),
  AV with a ones-augmented V giving unnormalized sa + row sums,
  normalize, GEMM2 row-parallel -> yT partial (fp16).
All matmul operands are fp16 (fp32 PSUM accumulate). Host shards inputs, sums
the 4 per-batch partials (the "all-reduce"), and adds the output bias fold
(bproj + bkqv_v @ Wproj - exact because softmax rows sum to 1).

v3 changes vs v2 (trace-driven):
  - exp activations widened to 1024-col PSUM chunks (2-bank sc tiles,
    double-buffered): ~48 fewer ACT instructions, ~10us less ACT busy
  - k/q bias adds moved DVE->ACT (activation Identity with per-partition
    bias AP); they run in phase A where ACT is otherwise idle
  - normalize multiply reads the AV PSUM tile directly (no uav SBUF
    evacuation): ~13us less DVE busy
  - strips emitted head-interleaved; GEMM2 emitted per-oc-chunk
  - filler pop scans past not-yet-ready entries; emits a dummy matmul if
    nothing is ready (a starved PE re-throttles the HAM clock gate)

v4 changes:
  - reciprocal_approx_fast CANNOT read PSUM on hardware (probe: garbage
    results; CoreSim disagrees) - row sum goes through an SBUF copy again
  - ALL matmuls contract over the full 128 partitions: q is stored
    zero-padded per head (qz0=[q0;0], qz1=[0;q1], qz2=[q2;0]), kT1 and
    saT2/wp2 are zero-padded too. Strip/GEMM2 stream cost is unchanged
    (cost = moving cols), but the PE activity monitor appears to weigh
    active rows: K=64-heavy phases ran at K=4/8 clock (1.2GHz) even when
    gap-free, K=128-heavy phases at 2.4GHz. 0*0=0 keeps results exact.
  - GEMM2 readiness keyed off actual AV pop times + normalize latency;
    drain phase keeps the PE fed with dummies while normalize chains run.

v6 changes:
  - exp ACT table preloaded via a tiny dummy activation at kernel start
    (the ~2.7us table load no longer delays the first real exp)
  - GEMM1 fused into the strip stream: only ci0-isl0 + the four ci1
    chains precede the first strip; remaining chains emitted inline just
    before the strips that consume them
  - AV groups split into <=4-matmul sub-chunks with per-chunk readiness
    gates (act watermark of the newest strip the chunk reads). Chunks of
    one group chain-gate each other (PSUM accumulation stays in order);
    groups release two-behind (only 2 AV PSUM bufs). The final AV group
    is now ~75% done before its last exp lands, shrinking the tail.

v7 changes (trace: startup was DMA-trigger-descriptor-generation bound,
~1.3-2.6us of engine time PER dma_start instruction at 128 descriptors):
  - inputs packed into 6 large DMAs (xT in 3 progressive splits, one wkq
    pack, two weight packs), all on the sync ring; ACT issues no DMAs so
    its stream is exps only
  - k/q bias adds back on DVE (tensor_scalar, f16 bias); ACT phase-B
    stream is now pure exp + the tail GEMM2 evacuations
  - zero-pad memsets moved to GpSimd (frees DVE at startup; library
    reload between Memset and PartitionBroadcast measured at only ~8ns)
  - hybrid strip order: group 0 and 3 head-blocked (early h2 readiness /
    short tail), groups 1-2 head-interleaved (spreads GEMM2 fillers)
  - GEMM2 group-3 evacuations on ACT (idle after the exp stream ends)

Self-contained: hardcodes all shapes; no sibling imports.
"""

import os

import numpy as np

B, N, D = 2, 2048, 768
H, HD = 12, 64
HPC = 3           # heads per core
NG = 4            # head groups
NCORES = 8
P = 128
NJ = N // P       # 16 j-chunks (keys) per head
NISL = 4          # 512-query i-slices

_compiled = None  # cached compiled Bass module
last_exec_time_ns = None
last_results = None

N_WARMUP = 9      # 512-wide dummy matmuls bridging boot -> first GEMM1


def _build():
    import concourse.tile as tile
    import concourse.mybir as mybir
    from concourse import bacc

    f32 = mybir.dt.float32
    f16 = mybir.dt.float16
    ADD = mybir.AluOpType.add
    MULT = mybir.AluOpType.mult
    EXP = mybir.ActivationFunctionType.Exp

    nc = bacc.Bacc(
        "TRN2", target_bir_lowering=False, debug=False, num_devices=NCORES
    )

    # packed DRAM layouts (see _host_prep). Few large DMAs: each dma_start
    # costs the issuing engine ~10-20ns per descriptor (= per partition
    # line), so 13 small loads wedge the ring for ~20us.
    xT_d = nc.dram_tensor("xTp", [P, NISL * 3072], f16, kind="ExternalInput").ap()
    wkq_d = nc.dram_tensor("wkqp", [P, 3 * 768], f16, kind="ExternalInput").ap()
    # wpackA: bkq(4, f16) | ident(128) | btri(128) | wv(1152)
    wpa_d = nc.dram_tensor("wpa", [P, 1412], f16, kind="ExternalInput").ap()
    # wpackB: wp01(768) | wp2(768)
    wpb_d = nc.dram_tensor("wpb", [P, 1536], f16, kind="ExternalInput").ap()
    yT_d = nc.dram_tensor("yT", [6, P, N], f16, kind="ExternalOutput").ap()

    yT_v = yT_d.rearrange("o p f -> p o f")      # [128, 6, 2048]

    with tile.TileContext(nc) as tc:
        import contextlib

        ctx = contextlib.ExitStack()
        with ctx:
            const = ctx.enter_context(tc.tile_pool(name="const", bufs=1))
            big = ctx.enter_context(tc.tile_pool(name="bigbufs", bufs=1))
            work = ctx.enter_context(tc.tile_pool(name="work", bufs=3))
            ypool = ctx.enter_context(tc.tile_pool(name="ypool", bufs=3))
            # PSUM budget (8 banks): sc 2x[128,1024] (4) + gemm 2x[128,512]
            # (2) + av 2x[65,512] (2)
            psum_sc = ctx.enter_context(
                tc.tile_pool(name="psum_sc", bufs=2, space="PSUM")
            )
            psum_gemm = ctx.enter_context(
                tc.tile_pool(name="psum_gemm", bufs=2, space="PSUM")
            )
            psum_av = ctx.enter_context(
                tc.tile_pool(name="psum_av", bufs=2, space="PSUM")
            )

            # ---- SBUF tiles ----
            xT_all = big.tile([P, NISL * 3072], f16, name="xT_all")
            wkq_all = const.tile([P, 3 * 768], f16, name="wkq_all")
            wpa = const.tile([P, 1412], f16, name="wpa")
            wpb = const.tile([P, 1536], f16, name="wpb")
            bkq_t = const.tile([P, 4], f32, name="bkq_t")
            ident_t = wpa[:, 4:132]
            btri_t = wpa[:, 132:260]
            wv_t = wpa[:, 260:1412]
            wp01_t = wpb[:, 0:768]
            wp2_t = wpb[:, 768:1536]
            wkq_ts = [wkq_all[:, 768 * c : 768 * c + 768] for c in range(3)]
            xT_ts = [xT_all[:, 3072 * i : 3072 * i + 3072] for i in range(NISL)]

            # 6 input DMAs, all on the sync ring, priority order: xT-isl0
            # gates the first GEMM1 chain; wpackA carries biases + the mask
            # + wv (early consumers); xT-isl1..3 gate the q projections;
            # wpackB (output projection) is needed last.
            nc.sync.dma_start(xT_ts[0], xT_d[:, 0:3072])
            nc.sync.dma_start(wkq_all[:], wkq_d)
            nc.sync.dma_start(wpa[:], wpa_d)
            nc.sync.dma_start(xT_ts[1], xT_d[:, 3072:6144])
            nc.sync.dma_start(xT_all[:, 6144:12288], xT_d[:, 6144:12288])
            nc.sync.dma_start(wpb[:], wpb_d)
            # biases arrive f16 inside wpackA; DVE wants f32 scalars
            nc.vector.tensor_copy(out=bkq_t[:], in_=wpa[:, 0:4])

            # PE warmup on a zeroed scratch while the first inputs land
            wscr = const.tile([P, 512], f16, name="wscr")
            nc.vector.memset(wscr[:], 0.0)

            # preload the exp ACT table so the ~2.7us load overlaps the
            # input DMA wait instead of delaying the first real exp
            tpre = const.tile([1, 8], f16, name="tpre")
            nc.scalar.activation(tpre[:], wscr[0:1, 0:8], EXP, scale=0.125)

            def emit_dummy():
                wps = psum_gemm.tile([P, 512], f32, tag="ps512", name="wps")
                nc.tensor.matmul(
                    wps[:, 0:512], wscr[:, 0:128], wscr[:], start=True, stop=True
                )

            for _ in range(N_WARMUP):
                emit_dummy()

            # k tiles: kT0 = [k0; k1] on 128 partitions, kT1 = [k2; zeros].
            # q tiles zero-padded per head so strip matmuls contract K=128:
            # qz0 = [q0; 0], qz1 = [0; q1], qz2 = [q2; 0]. The zero halves
            # contribute 0 to the scores; full-K keeps the PE clock warm.
            kT0 = big.tile([P, N], f16, name="kT0")
            kT1 = big.tile([P, N], f16, name="kT1")
            qzs = [big.tile([P, N], f16, name=f"qz{h}") for h in range(HPC)]
            q2st = big.tile([P, N], f16, name="q2st")
            # zero pads on GpSimd (keeps DVE free at startup; the one-off
            # Memset<->PartitionBroadcast library reload measures ~8ns)
            nc.gpsimd.memset(kT1[64:128, :], 0.0)
            nc.gpsimd.memset(qzs[0][64:128, :], 0.0)
            nc.gpsimd.memset(qzs[1][0:64, :], 0.0)
            nc.gpsimd.memset(qzs[2][64:128, :], 0.0)
            vaug = big.tile([P, NJ, HPC, 65], f16, name="vaug")
            nc.vector.memset(vaug[:, :, :, 64:65], 1.0)
            # saT: heads 0+1 packed on 128 partitions; head 2 zero-padded
            saT01s = [big.tile([P, 512], f16, name=f"saT01_{i}") for i in range(4)]
            saT2s = [big.tile([P, 512], f16, name=f"saT2_{i}") for i in range(4)]
            for i in range(4):
                nc.vector.memset(saT2s[i][64:128, :], 0.0)

            # ---- GEMM1 k/q: psum tile per (isl, ci) accumulated over dc ----
            # bias add + cast on ACT (idle during phase A)
            def emit_gemm1_kq(isl, ci):
                ps = psum_gemm.tile([P, 512], f32, tag="ps512", name="ps_kq")
                for dc in range(6):
                    nc.tensor.matmul(
                        ps[:, 0:512],
                        wkq_ts[ci][:, 128 * dc : 128 * dc + 128],
                        xT_ts[isl][:, 512 * dc : 512 * dc + 512],
                        start=(dc == 0),
                        stop=(dc == 5),
                    )
                sl = slice(512 * isl, 512 * isl + 512)
                if ci == 0:
                    nc.vector.tensor_scalar(
                        kT0[:, sl], ps[:, 0:512], bkq_t[:, 0:1], None, op0=ADD
                    )
                elif ci == 1:
                    nc.vector.tensor_scalar(
                        qzs[0][0:64, sl], ps[0:64, 0:512],
                        bkq_t[0:64, 1:2], None, op0=ADD,
                    )
                    nc.vector.tensor_scalar(
                        qzs[1][64:128, sl], ps[64:128, 0:512],
                        bkq_t[64:128, 1:2], None, op0=ADD,
                    )
                else:
                    nc.vector.tensor_scalar(
                        kT1[0:64, sl], ps[0:64, 0:512], bkq_t[0:64, 2:3],
                        None, op0=ADD,
                    )
                    nc.vector.tensor_scalar(
                        q2st[64:128, sl], ps[64:128, 0:512],
                        bkq_t[64:128, 3:4], None, op0=ADD,
                    )
                    nc.sync.dma_start(qzs[2][0:64, sl], q2st[64:128, sl])

            # ---- GEMM1 v: one psum tile per 128-query chunk ----
            def emit_gemm1_v(ic):
                ps = psum_gemm.tile([P, 512], f32, tag="ps512", name="ps_v")
                isl, k = divmod(ic, 4)
                for dc in range(6):
                    nc.tensor.matmul(
                        ps[:, 0:192],
                        xT_ts[isl][:, 512 * dc + 128 * k : 512 * dc + 128 * k + 128],
                        wv_t[:, 192 * dc : 192 * dc + 192],
                        start=(dc == 0),
                        stop=(dc == 5),
                    )
                nc.vector.tensor_copy(
                    out=vaug[:, ic, :, 0:64],
                    in_=ps[:, 0:192].rearrange("p (h d) -> p h d", h=HPC),
                )

            # ---- strips: scoresT + causal mask + exp (1024-wide chunks) ----
            all_strips = [[None] * NJ for _ in range(HPC)]

            def emit_strip(h, jc):
                kTc = kT0 if h < 2 else kT1
                qTc = qzs[h]
                i0 = 128 * jc
                W = N - i0
                strip = work.tile(
                    [P, W], f16, tag=f"expT{jc}", bufs=3, name=f"expT{jc}"
                )
                for c0 in range(0, W, 1024):
                    cw = min(1024, W - c0)
                    ps = psum_sc.tile([P, 1024], f32, tag="sc", name="ps_s")
                    for s0 in range(c0, c0 + cw, 512):
                        sw = min(512, W - s0)
                        chained = s0 == 0
                        nc.tensor.matmul(
                            ps[:, s0 - c0 : s0 - c0 + sw],
                            kTc[:, i0 : i0 + 128],
                            qTc[:, i0 + s0 : i0 + s0 + sw],
                            start=True,
                            stop=(not chained),
                        )
                        if chained:
                            # causal mask: accumulate -30000 above the diagonal
                            nc.tensor.matmul(
                                ps[:, 0:128], ident_t[:], btri_t[:],
                                start=False, stop=True,
                            )
                    nc.scalar.activation(
                        strip[:, c0 : c0 + cw], ps[:, 0:cw], EXP, scale=0.125
                    )
                all_strips[h][jc] = strip

            # ---- AV (emitted in sub-chunks) + normalize ----
            av_ps = {}

            def emit_av_part(h, iseg, jlo, jhi):
                strips = all_strips[h]
                jmax = 4 * iseg + 3
                if jlo == 0:
                    av_ps[(h, iseg)] = psum_av.tile(
                        [65, 512], f32, tag="av", name="ps2"
                    )
                ps2 = av_ps[(h, iseg)]
                for jc in range(jlo, jhi + 1):
                    off = 512 * iseg - 128 * jc
                    lo = max(0, off)
                    w = 512 - (lo - off)
                    nc.tensor.matmul(
                        ps2[0:65, 512 - w : 512],
                        vaug[:, jc, h, :],
                        strips[jc][:, lo : lo + w],
                        start=(jc == 0),
                        stop=(jc == jmax),
                    )
                if jhi != jmax:
                    return
                # row sum must bounce through SBUF: reciprocal_approx_fast
                # reads garbage from PSUM on hardware. The multiply below can
                # read PSUM directly. GpSimd runs ONLY partition_broadcast
                # (mixing op types forces library reloads).
                srow = work.tile([1, 512], f32, tag="srow", bufs=2, name="srow")
                nc.vector.tensor_copy(out=srow[:], in_=ps2[64:65, :])
                rrow = work.tile([1, 512], f32, tag="rrow", bufs=2, name="rrow")
                nc.vector.reciprocal_approx_fast(out=rrow[:], in_=srow[:])
                rbc = work.tile([64, 512], f32, tag="rbc", bufs=2, name="rbc")
                nc.gpsimd.partition_broadcast(rbc[:], rrow[:])
                if h == 0:
                    nc.vector.tensor_tensor(
                        saT01s[iseg][0:64, :], ps2[0:64, :], rbc[:], MULT
                    )
                elif h == 1:
                    st1 = work.tile([64, 512], f16, tag="st1", bufs=2, name="st1")
                    nc.vector.tensor_tensor(st1[:], ps2[0:64, :], rbc[:], MULT)
                    nc.sync.dma_start(saT01s[iseg][64:128, :], st1[:])
                else:
                    nc.vector.tensor_tensor(
                        saT2s[iseg][0:64, :], ps2[0:64, :], rbc[:], MULT
                    )

            # ---- GEMM2: both matmuls contract 128 (wp2/saT2 zero-padded) ----
            def emit_gemm2_oc(isl, oc):
                ps = psum_gemm.tile([P, 512], f32, tag="ps512", name="ps_y")
                nc.tensor.matmul(
                    ps[:, 0:512],
                    wp01_t[:, 128 * oc : 128 * oc + 128],
                    saT01s[isl][:],
                    start=True,
                    stop=False,
                )
                nc.tensor.matmul(
                    ps[:, 0:512],
                    wp2_t[:, 128 * oc : 128 * oc + 128],
                    saT2s[isl][:, :],
                    start=False,
                    stop=True,
                )
                yst = ypool.tile([P, 512], f16, tag="yst", name="yst")
                if isl == 3:
                    # group-3 evacuations on ACT: it is idle once the exp
                    # stream has drained, while DVE still runs normalizes
                    nc.scalar.copy(yst[:], ps[:, 0:512])
                else:
                    nc.vector.tensor_copy(out=yst[:], in_=ps[:, 0:512])
                nc.sync.dma_start(
                    yT_v[:, oc, 512 * isl : 512 * isl + 512], yst[:]
                )

            # ---- emission schedule ----
            # Costs in ns for the pacing model (warm clock).
            def strip_pe_cost(W):
                return W / 2.4 + 110 * ((W + 511) // 512) + 160

            def strip_act_cost(W):
                # calibrated: measured exp busy = 0.833ns/col + ~210ns/chunk
                return 0.833 * W + 210 * ((W + 1023) // 1024)

            # fillers: mutable [ready_gate, pe_cost, emit_fn] entries. The
            # gate compares against pe_t (emitted-PE-work watermark).
            fillers = []
            for ic in range(16):
                fillers.append([0.0, 580.0, lambda ic=ic: emit_gemm1_v(ic)])

            pe_t = 0.0    # PE-busy time emitted so far (phase B origin)
            act_t = 0.0   # ACT-busy time emitted so far
            SLACK = 3000.0
            NORM_DELAY = 3000.0   # AV drain -> saT ready (recip+bcast+mult)
            CHAIN = 6 * 512 / 2.4 + 120   # one GEMM1 chain on PE
            n_dummy = 0
            act_after = {}        # (h, jc) -> act_t watermark after its exp

            def emit_chain(isl, ci):
                # GEMM1 chain emitted inline in the strip stream
                nonlocal pe_t, act_t
                emit_gemm1_kq(isl, ci)
                pe_t += CHAIN
                act_t += 700.0 * (1 if ci == 0 else 2)

            def pop_fillers(budget, allow_dummy=True):
                # Pop ready fillers (scanning past not-yet-ready ones; AV
                # chunk/group ordering is enforced via dynamic gates). If
                # nothing is ready and a real deficit remains, emit a dummy
                # matmul: a starved PE re-throttles the HAM clock.
                nonlocal pe_t, n_dummy
                spent = 0.0
                while fillers and spent < budget:
                    for i, e in enumerate(fillers):
                        if e[0] <= pe_t:
                            fillers.pop(i)
                            e[2]()
                            pe_t += e[1]
                            spent += e[1]
                            break
                    else:
                        if allow_dummy and budget - spent > 400.0 and n_dummy < 150:
                            emit_dummy()
                            n_dummy += 1
                            pe_t += 215.0
                            spent += 215.0
                        else:
                            break
                return spent

            # AV groups: sub-chunks of <=4 jcs. Chunk i+1's gate opens when
            # chunk i pops (PSUM chain order); a group's first chunk opens
            # when the group two-before finished (2 AV PSUM bufs). GEMM2(g)
            # opens after AV(2,g)'s last chunk + normalize latency.
            av_ord = 0            # append ordinal
            av_group_done = set() # ordinals whose last chunk popped
            av_pending_first = {} # ordinal -> (entry, act_gate)
            av_appended = {}      # g -> number of AV heads appended
            av_done_count = {}    # g -> number of AV heads fully popped
            gemm2_entries = {}

            def append_av_group(h, g):
                nonlocal av_ord
                n = av_ord
                av_ord += 1
                jmax = 4 * g + 3
                parts = [(jlo, min(jlo + 3, jmax)) for jlo in range(0, jmax + 1, 4)]
                entries = []
                for idx, (jlo, jhi) in enumerate(parts):
                    cols = sum(
                        min(512, 512 + 512 * g - 128 * jc)
                        for jc in range(jlo, jhi + 1)
                    )
                    cost = cols / 2.4 + 40.0 * (jhi - jlo + 1)
                    last = jhi == jmax

                    def fn(h=h, g=g, jlo=jlo, jhi=jhi, idx=idx, n=n,
                           cost=cost, last=last):
                        emit_av_part(h, g, jlo, jhi)
                        if idx + 1 < len(entries):
                            # open the next chunk of this group
                            entries[idx + 1][0] = gates[idx + 1]
                        if last:
                            av_group_done.add(n)
                            # release the group two ahead (2 AV PSUM bufs)
                            if n + 2 in av_pending_first:
                                e2, gate2 = av_pending_first.pop(n + 2)
                                e2[0] = gate2
                            av_done_count[g] = av_done_count.get(g, 0) + 1
                            if av_done_count[g] == HPC:
                                # all 3 heads' saT ready soon: open GEMM2
                                for e2 in gemm2_entries[g]:
                                    e2[0] = pe_t + cost + NORM_DELAY

                    entries.append([float("inf"), cost, fn])
                gates = [act_after[(h, jhi)] + SLACK for (jlo, jhi) in parts]
                # first chunk: open if the group two-before is done
                if n < 2 or (n - 2) in av_group_done:
                    entries[0][0] = gates[0]
                else:
                    av_pending_first[n] = (entries[0], gates[0])
                if g not in gemm2_entries:
                    gemm2_entries[g] = [
                        [float("inf"), 620.0,
                         lambda g=g, oc=oc: emit_gemm2_oc(g, oc)]
                        for oc in range(6)
                    ]
                fillers.extend(entries)
                av_appended[g] = av_appended.get(g, 0) + 1
                if av_appended[g] == HPC:
                    fillers.extend(gemm2_entries[g])

            # ---- prefix: enough GEMM1 for the first strips ----
            emit_chain(0, 0)               # kT0 block 0 (g0, h0/h1)
            for isl in range(NISL):
                emit_chain(isl, 1)         # qz0/qz1 complete
            # reset pacing origin at the start of the strip stream
            pe_t = 0.0
            act_t = 0.0

            # hybrid strip order:
            #  group 0 head-blocked, ci2 chains inlined among the h0/h1
            #    strips (all four must precede the first h2 strip: qz2 is
            #    streamed to full N)
            #  groups 1-2 head-interleaved (AV/GEMM2 fillers spread evenly)
            #  group 3 head-blocked (AV(0,3)/AV(1,3) finish early, so the
            #    final AV(2,3) is ~75% done before its last exp; short tail)
            strip_order = []
            for jc in range(4):
                strip_order += [(0, jc)]
            for jc in range(4):
                strip_order += [(1, jc)]
            for jc in range(4):
                strip_order += [(2, jc)]
            for jc in range(4, 12):
                strip_order += [(h, jc) for h in range(HPC)]
            for h in (2, 0, 1):
                strip_order += [(h, jc) for jc in range(12, 16)]

            # inline GEMM1 chains (isl, ci) before their first consumer
            inline_chains = {
                (0, 1): [(0, 2)],   # ci2-isl0
                (0, 3): [(1, 2)],   # ci2-isl1
                (1, 1): [(2, 2)],   # ci2-isl2
                (1, 3): [(3, 2)],   # ci2-isl3  (before first h2 strip)
                (0, 4): [(1, 0)],   # kT0 block for group 1
                (0, 8): [(2, 0)],   # kT0 block for group 2
                (0, 12): [(3, 0)],  # kT0 block for group 3
            }

            for h, jc in strip_order:
                for isl, ci in inline_chains.get((h, jc), []):
                    emit_chain(isl, ci)
                W = N - 128 * jc
                emit_strip(h, jc)
                pe_t += strip_pe_cost(W)
                act_t += strip_act_cost(W)
                act_after[(h, jc)] = act_t
                if jc % 4 == 3:
                    append_av_group(h, jc // 4)
                # keep PE slightly ahead of ACT but not idle
                pop_fillers(act_t - pe_t)

            # drain: keep popping; feed dummies while gates (normalize
            # chains) are still closed, then force-pop in order
            while fillers:
                if pop_fillers(1e9, allow_dummy=False) == 0.0:
                    if n_dummy < 150:
                        emit_dummy()
                        n_dummy += 1
                        pe_t += 215.0
                    else:
                        e = fillers.pop(0)
                        e[2]()
                        pe_t += e[1]

    nc.compile()
    return nc


def _host_prep(x, Wkqv, bkqv, Wproj, bproj):
    f16 = np.float16
    Wk = Wkqv[:, 0:D]
    Wq = Wkqv[:, D : 2 * D]
    Wv = Wkqv[:, 2 * D : 3 * D]
    bk = bkqv[0:D]
    bq = bkqv[D : 2 * D]
    bv = bkqv[2 * D : 3 * D]
    out_bias = (bproj + bv @ Wproj).astype(np.float32)  # softmax rows sum to 1

    ident = np.eye(P, dtype=f16)
    # btri[k, i] = -30000 where k > i: accumulated into scoresT diag blocks,
    # exp((s - 30000) * 0.125) underflows to exactly 0 in fp16.
    btri = (np.tril(np.full((P, P), -30000.0, np.float32), -1)).astype(f16)

    in_maps = []
    for b in range(B):
        xT = x[b].T.astype(f16)                       # [768, 2048]
        # [pi, 3072*isl + 512*dc + c] = xT[128*dc + pi, 512*isl + c]
        xTp = np.ascontiguousarray(
            xT.reshape(6, P, NISL, 512).transpose(1, 2, 0, 3).reshape(P, NISL * 3072)
        )
        for g in range(NG):
            hs = [HPC * g + i for i in range(HPC)]
            wk = [np.asarray(Wk[:, HD * h : HD * h + HD]) for h in hs]
            wq = [np.asarray(Wq[:, HD * h : HD * h + HD]) for h in hs]
            wv = [np.asarray(Wv[:, HD * h : HD * h + HD]) for h in hs]
            # column chunks: ci0 = k01, ci1 = q01, ci2 = k2|q2
            wkq = np.concatenate(
                [wk[0], wk[1], wq[0], wq[1], wk[2], wq[2]], axis=1
            ).astype(np.float32)                       # [768, 384]
            # [pi, 768*ci + 128*dc + c] = wkq[128*dc + pi, 128*ci + c]
            wkqp = np.ascontiguousarray(
                wkq.reshape(6, P, 3, P).transpose(1, 2, 0, 3).reshape(P, 3 * 768)
            ).astype(f16)
            wv_c = np.concatenate(wv, axis=1).astype(np.float32)   # [768, 192]
            # [pi, dc*192 + c] = wv_c[128*dc + pi, c]
            wvp = np.ascontiguousarray(
                wv_c.reshape(6, P, 192).transpose(1, 0, 2).reshape(P, 6 * 192)
            ).astype(f16)
            wp01 = np.concatenate(
                [Wproj[HD * hs[0] : HD * hs[0] + HD, :],
                 Wproj[HD * hs[1] : HD * hs[1] + HD, :]], axis=0
            ).astype(f16)                              # [128, 768]
            wp2 = np.zeros((P, D), f16)                # [128, 768], rows 64+ zero
            wp2[0:64, :] = Wproj[HD * hs[2] : HD * hs[2] + HD, :].astype(f16)
            bkq = np.zeros((P, 4), f16)
            bkq[:, 0] = np.concatenate(
                [bk[HD * hs[0] : HD * hs[0] + HD], bk[HD * hs[1] : HD * hs[1] + HD]]
            )
            bkq[:, 1] = np.concatenate(
                [bq[HD * hs[0] : HD * hs[0] + HD], bq[HD * hs[1] : HD * hs[1] + HD]]
            )
            bkq[0:64, 2] = bk[HD * hs[2] : HD * hs[2] + HD]
            bkq[64:128, 3] = bq[HD * hs[2] : HD * hs[2] + HD]
            # wpackA: bkq | ident | btri | wv;  wpackB: wp01 | wp2
            wpa = np.concatenate([bkq, ident, btri, wvp], axis=1)
            wpb = np.concatenate([wp01, wp2], axis=1)
            in_maps.append(dict(xTp=xTp, wkqp=wkqp, wpa=wpa, wpb=wpb))
    return in_maps, out_bias


def kernel(x, Wkqv, bkqv, Wproj, bproj):
    global _compiled, last_exec_time_ns, last_results
    import concourse.bass_utils as bass_utils

    x = np.asarray(x, np.float32)
    Wkqv = np.asarray(Wkqv, np.float32)
    bkqv = np.asarray(bkqv, np.float32)
    Wproj = np.asarray(Wproj, np.float32)
    bproj = np.asarray(bproj, np.float32)

    if _compiled is None:
        _compiled = _build()
    nc = _compiled

    in_maps, out_bias = _host_prep(x, Wkqv, bkqv, Wproj, bproj)

    trace = os.environ.get("BASS_KERNEL_TRACE", "0") == "1"
    res = bass_utils.run_bass_kernel_spmd(
        nc, in_maps, core_ids=list(range(NCORES)), trace=trace
    )
    last_exec_time_ns = res.exec_time_ns
    last_results = res

    out = np.zeros((B, N, D), np.float32)
    for b in range(B):
        acc = np.zeros((D, N), np.float32)
        for g in range(NG):
            acc += res.results[b * NG + g]["yT"].reshape(D, N).astype(np.float32)
        out[b] = acc.T + out_bias
    return out


# revision 43
# speedup vs baseline: 1.1190x; 1.1190x over previous
"""Causal self-attention (B=2, N=2048, D=768, H=12) on 8 Trainium2 NeuronCores.

Sharding: data-parallel over batch (2) x tensor-parallel over head groups (4),
3 heads per core. Each core computes, for its (batch, head-group):
  GEMM1: kT/qT (transposed) and v (natural) projections from xT,
  scores^T = k @ q^T per head, exp on ScalarE (fp16 out),
  AV with a ones-augmented V giving unnormalized sa + row sums,
  normalize, GEMM2 row-parallel -> yT partial (fp16).
All matmul operands are fp16 (fp32 PSUM accumulate). Host shards inputs, sums
the 4 per-batch partials (the "all-reduce"), and adds the output bias fold
(bproj + bkqv_v @ Wproj - exact because softmax rows sum to 1).

v3 changes vs v2 (trace-driven):
  - exp activations widened to 1024-col PSUM chunks (2-bank sc tiles,
    double-buffered): ~48 fewer ACT instructions, ~10us less ACT busy
  - k/q bias adds moved DVE->ACT (activation Identity with per-partition
    bias AP); they run in phase A where ACT is otherwise idle
  - normalize multiply reads the AV PSUM tile directly (no uav SBUF
    evacuation): ~13us less DVE busy
  - strips emitted head-interleaved; GEMM2 emitted per-oc-chunk
  - filler pop scans past not-yet-ready entries; emits a dummy matmul if
    nothing is ready (a starved PE re-throttles the HAM clock gate)

v4 changes:
  - reciprocal_approx_fast CANNOT read PSUM on hardware (probe: garbage
    results; CoreSim disagrees) - row sum goes through an SBUF copy again
  - ALL matmuls contract over the full 128 partitions: q is stored
    zero-padded per head (qz0=[q0;0], qz1=[0;q1], qz2=[q2;0]), kT1 and
    saT2/wp2 are zero-padded too. Strip/GEMM2 stream cost is unchanged
    (cost = moving cols), but the PE activity monitor appears to weigh
    active rows: K=64-heavy phases ran at K=4/8 clock (1.2GHz) even when
    gap-free, K=128-heavy phases at 2.4GHz. 0*0=0 keeps results exact.
  - GEMM2 readiness keyed off actual AV pop times + normalize latency;
    drain phase keeps the PE fed with dummies while normalize chains run.

v6 changes:
  - exp ACT table preloaded via a tiny dummy activation at kernel start
    (the ~2.7us table load no longer delays the first real exp)
  - GEMM1 fused into the strip stream: only ci0-isl0 + the four ci1
    chains precede the first strip; remaining chains emitted inline just
    before the strips that consume them
  - AV groups split into <=4-matmul sub-chunks with per-chunk readiness
    gates (act watermark of the newest strip the chunk reads). Chunks of
    one group chain-gate each other (PSUM accumulation stays in order);
    groups release two-behind (only 2 AV PSUM bufs). The final AV group
    is now ~75% done before its last exp lands, shrinking the tail.

v7 changes (trace: startup was DMA-trigger-descriptor-generation bound,
~1.3-2.6us of engine time PER dma_start instruction at 128 descriptors):
  - inputs packed into 6 large DMAs (xT in 3 progressive splits, one wkq
    pack, two weight packs), all on the sync ring; ACT issues no DMAs so
    its stream is exps only
  - k/q bias adds stay on ACT (moving them to DVE inflated every exp
    ~190ns - concurrent-engine SBUF contention; measured and reverted)
  - zero-pad memsets moved to GpSimd (frees DVE at startup; library
    reload between Memset and PartitionBroadcast measured at only ~8ns)
  - hybrid strip order: group 0 and 3 head-blocked (early h2 readiness /
    short tail), groups 1-2 head-interleaved (spreads GEMM2 fillers)
  - GEMM2 group-3 evacuations on ACT (idle after the exp stream ends)

Self-contained: hardcodes all shapes; no sibling imports.
"""

import os

import numpy as np

B, N, D = 2, 2048, 768
H, HD = 12, 64
HPC = 3           # heads per core
NG = 4            # head groups
NCORES = 8
P = 128
NJ = N // P       # 16 j-chunks (keys) per head
NISL = 4          # 512-query i-slices

_compiled = None  # cached compiled Bass module
last_exec_time_ns = None
last_results = None

N_WARMUP = 9      # 512-wide dummy matmuls bridging boot -> first GEMM1


def _build():
    import concourse.tile as tile
    import concourse.mybir as mybir
    from concourse import bacc

    f32 = mybir.dt.float32
    f16 = mybir.dt.float16
    ADD = mybir.AluOpType.add
    MULT = mybir.AluOpType.mult
    EXP = mybir.ActivationFunctionType.Exp

    nc = bacc.Bacc(
        "TRN2", target_bir_lowering=False, debug=False, num_devices=NCORES
    )

    # packed DRAM layouts (see _host_prep). Few large DMAs: each dma_start
    # costs the issuing engine ~10-20ns per descriptor (= per partition
    # line), so 13 small loads wedge the ring for ~20us.
    xT_d = nc.dram_tensor("xTp", [P, NISL * 3072], f16, kind="ExternalInput").ap()
    wkq_d = nc.dram_tensor("wkqp", [P, 3 * 768], f16, kind="ExternalInput").ap()
    # wpackA: bkq(4, f16) | ident(128) | btri(128) | wv(1152)
    wpa_d = nc.dram_tensor("wpa", [P, 1412], f16, kind="ExternalInput").ap()
    # wpackB: wp01(768) | wp2(768)
    wpb_d = nc.dram_tensor("wpb", [P, 1536], f16, kind="ExternalInput").ap()
    yT_d = nc.dram_tensor("yT", [6, P, N], f16, kind="ExternalOutput").ap()

    yT_v = yT_d.rearrange("o p f -> p o f")      # [128, 6, 2048]

    with tile.TileContext(nc) as tc:
        import contextlib

        ctx = contextlib.ExitStack()
        with ctx:
            const = ctx.enter_context(tc.tile_pool(name="const", bufs=1))
            big = ctx.enter_context(tc.tile_pool(name="bigbufs", bufs=1))
            work = ctx.enter_context(tc.tile_pool(name="work", bufs=3))
            ypool = ctx.enter_context(tc.tile_pool(name="ypool", bufs=3))
            # PSUM budget (8 banks): sc 2x[128,1024] (4) + gemm 2x[128,512]
            # (2) + av 2x[65,512] (2)
            psum_sc = ctx.enter_context(
                tc.tile_pool(name="psum_sc", bufs=2, space="PSUM")
            )
            psum_gemm = ctx.enter_context(
                tc.tile_pool(name="psum_gemm", bufs=2, space="PSUM")
            )
            psum_av = ctx.enter_context(
                tc.tile_pool(name="psum_av", bufs=2, space="PSUM")
            )

            # ---- SBUF tiles ----
            xT_all = big.tile([P, NISL * 3072], f16, name="xT_all")
            wkq_all = const.tile([P, 3 * 768], f16, name="wkq_all")
            wpa = const.tile([P, 1412], f16, name="wpa")
            wpb = const.tile([P, 1536], f16, name="wpb")
            bkq_t = const.tile([P, 4], f32, name="bkq_t")
            ident_t = wpa[:, 4:132]
            btri_t = wpa[:, 132:260]
            wv_t = wpa[:, 260:1412]
            wp01_t = wpb[:, 0:768]
            wp2_t = wpb[:, 768:1536]
            wkq_ts = [wkq_all[:, 768 * c : 768 * c + 768] for c in range(3)]
            xT_ts = [xT_all[:, 3072 * i : 3072 * i + 3072] for i in range(NISL)]

            # 6 input DMAs, all on the sync ring, priority order: xT-isl0
            # gates the first GEMM1 chain; wpackA carries biases + the mask
            # + wv (early consumers); xT-isl1..3 gate the q projections;
            # wpackB (output projection) is needed last.
            nc.sync.dma_start(xT_ts[0], xT_d[:, 0:3072])
            nc.sync.dma_start(wkq_all[:], wkq_d)
            nc.sync.dma_start(wpa[:], wpa_d)
            nc.sync.dma_start(xT_ts[1], xT_d[:, 3072:6144])
            nc.sync.dma_start(xT_all[:, 6144:12288], xT_d[:, 6144:12288])
            nc.sync.dma_start(wpb[:], wpb_d)
            # biases arrive f16 inside wpackA; DVE wants f32 scalars
            nc.vector.tensor_copy(out=bkq_t[:], in_=wpa[:, 0:4])

            # PE warmup on a zeroed scratch while the first inputs land
            wscr = const.tile([P, 512], f16, name="wscr")
            nc.vector.memset(wscr[:], 0.0)

            # preload the exp ACT table so the ~2.7us load overlaps the
            # input DMA wait instead of delaying the first real exp
            tpre = const.tile([1, 8], f16, name="tpre")
            nc.scalar.activation(tpre[:], wscr[0:1, 0:8], EXP, scale=0.125)

            def emit_dummy():
                wps = psum_gemm.tile([P, 512], f32, tag="ps512", name="wps")
                nc.tensor.matmul(
                    wps[:, 0:512], wscr[:, 0:128], wscr[:], start=True, stop=True
                )

            for _ in range(N_WARMUP):
                emit_dummy()

            # k tiles: kT0 = [k0; k1] on 128 partitions, kT1 = [k2; zeros].
            # q tiles zero-padded per head so strip matmuls contract K=128:
            # qz0 = [q0; 0], qz1 = [0; q1], qz2 = [q2; 0]. The zero halves
            # contribute 0 to the scores; full-K keeps the PE clock warm.
            kT0 = big.tile([P, N], f16, name="kT0")
            kT1 = big.tile([P, N], f16, name="kT1")
            qzs = [big.tile([P, N], f16, name=f"qz{h}") for h in range(HPC)]
            q2st = big.tile([P, N], f16, name="q2st")
            # zero pads on GpSimd (keeps DVE free at startup; the one-off
            # Memset<->PartitionBroadcast library reload measures ~8ns)
            nc.gpsimd.memset(kT1[64:128, :], 0.0)
            nc.gpsimd.memset(qzs[0][64:128, :], 0.0)
            nc.gpsimd.memset(qzs[1][0:64, :], 0.0)
            nc.gpsimd.memset(qzs[2][64:128, :], 0.0)
            vaug = big.tile([P, NJ, HPC, 65], f16, name="vaug")
            nc.vector.memset(vaug[:, :, :, 64:65], 1.0)
            # saT: heads 0+1 packed on 128 partitions; head 2 zero-padded
            saT01s = [big.tile([P, 512], f16, name=f"saT01_{i}") for i in range(4)]
            saT2s = [big.tile([P, 512], f16, name=f"saT2_{i}") for i in range(4)]
            for i in range(4):
                nc.vector.memset(saT2s[i][64:128, :], 0.0)

            # ---- GEMM1 k/q: psum tile per (isl, ci) accumulated over dc ----
            # bias add + cast on ACT (idle during phase A)
            def emit_gemm1_kq(isl, ci):
                ps = psum_gemm.tile([P, 512], f32, tag="ps512", name="ps_kq")
                for dc in range(6):
                    nc.tensor.matmul(
                        ps[:, 0:512],
                        wkq_ts[ci][:, 128 * dc : 128 * dc + 128],
                        xT_ts[isl][:, 512 * dc : 512 * dc + 512],
                        start=(dc == 0),
                        stop=(dc == 5),
                    )
                sl = slice(512 * isl, 512 * isl + 512)
                if ci == 0:
                    nc.scalar.add(kT0[:, sl], ps[:, 0:512], bkq_t[:, 0:1])
                elif ci == 1:
                    nc.scalar.add(
                        qzs[0][0:64, sl], ps[0:64, 0:512], bkq_t[0:64, 1:2]
                    )
                    nc.scalar.add(
                        qzs[1][64:128, sl], ps[64:128, 0:512], bkq_t[64:128, 1:2]
                    )
                else:
                    nc.scalar.add(
                        kT1[0:64, sl], ps[0:64, 0:512], bkq_t[0:64, 2:3]
                    )
                    nc.scalar.add(
                        q2st[64:128, sl], ps[64:128, 0:512], bkq_t[64:128, 3:4]
                    )
                    nc.sync.dma_start(qzs[2][0:64, sl], q2st[64:128, sl])

            # ---- GEMM1 v: one psum tile per 128-query chunk ----
            def emit_gemm1_v(ic):
                ps = psum_gemm.tile([P, 512], f32, tag="ps512", name="ps_v")
                isl, k = divmod(ic, 4)
                for dc in range(6):
                    nc.tensor.matmul(
                        ps[:, 0:192],
                        xT_ts[isl][:, 512 * dc + 128 * k : 512 * dc + 128 * k + 128],
                        wv_t[:, 192 * dc : 192 * dc + 192],
                        start=(dc == 0),
                        stop=(dc == 5),
                    )
                nc.vector.tensor_copy(
                    out=vaug[:, ic, :, 0:64],
                    in_=ps[:, 0:192].rearrange("p (h d) -> p h d", h=HPC),
                )

            # ---- strips: scoresT + causal mask + exp (1024-wide chunks) ----
            all_strips = [[None] * NJ for _ in range(HPC)]

            def emit_strip(h, jc):
                kTc = kT0 if h < 2 else kT1
                qTc = qzs[h]
                i0 = 128 * jc
                W = N - i0
                strip = work.tile(
                    [P, W], f16, tag=f"expT{jc}", bufs=3, name=f"expT{jc}"
                )
                for c0 in range(0, W, 1024):
                    cw = min(1024, W - c0)
                    ps = psum_sc.tile([P, 1024], f32, tag="sc", name="ps_s")
                    for s0 in range(c0, c0 + cw, 512):
                        sw = min(512, W - s0)
                        chained = s0 == 0
                        nc.tensor.matmul(
                            ps[:, s0 - c0 : s0 - c0 + sw],
                            kTc[:, i0 : i0 + 128],
                            qTc[:, i0 + s0 : i0 + s0 + sw],
                            start=True,
                            stop=(not chained),
                        )
                        if chained:
                            # causal mask: accumulate -30000 above the diagonal
                            nc.tensor.matmul(
                                ps[:, 0:128], ident_t[:], btri_t[:],
                                start=False, stop=True,
                            )
                    nc.scalar.activation(
                        strip[:, c0 : c0 + cw], ps[:, 0:cw], EXP, scale=0.125
                    )
                all_strips[h][jc] = strip

            # ---- AV (emitted in sub-chunks) + normalize ----
            av_ps = {}

            def emit_av_part(h, iseg, jlo, jhi):
                strips = all_strips[h]
                jmax = 4 * iseg + 3
                if jlo == 0:
                    av_ps[(h, iseg)] = psum_av.tile(
                        [65, 512], f32, tag="av", name="ps2"
                    )
                ps2 = av_ps[(h, iseg)]
                for jc in range(jlo, jhi + 1):
                    off = 512 * iseg - 128 * jc
                    lo = max(0, off)
                    w = 512 - (lo - off)
                    nc.tensor.matmul(
                        ps2[0:65, 512 - w : 512],
                        vaug[:, jc, h, :],
                        strips[jc][:, lo : lo + w],
                        start=(jc == 0),
                        stop=(jc == jmax),
                    )
                if jhi != jmax:
                    return
                # row sum must bounce through SBUF: reciprocal_approx_fast
                # reads garbage from PSUM on hardware. The multiply below can
                # read PSUM directly. GpSimd runs ONLY partition_broadcast
                # (mixing op types forces library reloads).
                srow = work.tile([1, 512], f32, tag="srow", bufs=2, name="srow")
                nc.vector.tensor_copy(out=srow[:], in_=ps2[64:65, :])
                rrow = work.tile([1, 512], f32, tag="rrow", bufs=2, name="rrow")
                nc.vector.reciprocal_approx_fast(out=rrow[:], in_=srow[:])
                rbc = work.tile([64, 512], f32, tag="rbc", bufs=2, name="rbc")
                nc.gpsimd.partition_broadcast(rbc[:], rrow[:])
                if h == 0:
                    nc.vector.tensor_tensor(
                        saT01s[iseg][0:64, :], ps2[0:64, :], rbc[:], MULT
                    )
                elif h == 1:
                    st1 = work.tile([64, 512], f16, tag="st1", bufs=2, name="st1")
                    nc.vector.tensor_tensor(st1[:], ps2[0:64, :], rbc[:], MULT)
                    nc.sync.dma_start(saT01s[iseg][64:128, :], st1[:])
                else:
                    nc.vector.tensor_tensor(
                        saT2s[iseg][0:64, :], ps2[0:64, :], rbc[:], MULT
                    )

            # ---- GEMM2: both matmuls contract 128 (wp2/saT2 zero-padded) ----
            def emit_gemm2_oc(isl, oc):
                ps = psum_gemm.tile([P, 512], f32, tag="ps512", name="ps_y")
                nc.tensor.matmul(
                    ps[:, 0:512],
                    wp01_t[:, 128 * oc : 128 * oc + 128],
                    saT01s[isl][:],
                    start=True,
                    stop=False,
                )
                nc.tensor.matmul(
                    ps[:, 0:512],
                    wp2_t[:, 128 * oc : 128 * oc + 128],
                    saT2s[isl][:, :],
                    start=False,
                    stop=True,
                )
                yst = ypool.tile([P, 512], f16, tag="yst", name="yst")
                if isl == 3:
                    # group-3 evacuations on ACT: it is idle once the exp
                    # stream has drained, while DVE still runs normalizes
                    nc.scalar.copy(yst[:], ps[:, 0:512])
                else:
                    nc.vector.tensor_copy(out=yst[:], in_=ps[:, 0:512])
                nc.sync.dma_start(
                    yT_v[:, oc, 512 * isl : 512 * isl + 512], yst[:]
                )

            # ---- emission schedule ----
            # Costs in ns for the pacing model (warm clock).
            def strip_pe_cost(W):
                return W / 2.4 + 110 * ((W + 511) // 512) + 160

            def strip_act_cost(W):
                # calibrated: measured exp busy = 0.833ns/col + ~210ns/chunk
                return 0.833 * W + 210 * ((W + 1023) // 1024)

            # fillers: mutable [ready_gate, pe_cost, emit_fn] entries. The
            # gate compares against pe_t (emitted-PE-work watermark).
            fillers = []
            for ic in range(16):
                fillers.append([0.0, 580.0, lambda ic=ic: emit_gemm1_v(ic)])

            pe_t = 0.0    # PE-busy time emitted so far (phase B origin)
            act_t = 0.0   # ACT-busy time emitted so far
            SLACK = 3000.0
            NORM_DELAY = 3000.0   # AV drain -> saT ready (recip+bcast+mult)
            CHAIN = 6 * 512 / 2.4 + 120   # one GEMM1 chain on PE
            n_dummy = 0
            act_after = {}        # (h, jc) -> act_t watermark after its exp

            def emit_chain(isl, ci):
                # GEMM1 chain emitted inline in the strip stream
                nonlocal pe_t, act_t
                emit_gemm1_kq(isl, ci)
                pe_t += CHAIN
                act_t += 700.0 * (1 if ci == 0 else 2)

            def pop_fillers(budget, allow_dummy=True):
                # Pop ready fillers (scanning past not-yet-ready ones; AV
                # chunk/group ordering is enforced via dynamic gates). If
                # nothing is ready and a real deficit remains, emit a dummy
                # matmul: a starved PE re-throttles the HAM clock.
                nonlocal pe_t, n_dummy
                spent = 0.0
                while fillers and spent < budget:
                    for i, e in enumerate(fillers):
                        if e[0] <= pe_t:
                            fillers.pop(i)
                            e[2]()
                            pe_t += e[1]
                            spent += e[1]
                            break
                    else:
                        if allow_dummy and budget - spent > 400.0 and n_dummy < 80:
                            emit_dummy()
                            n_dummy += 1
                            pe_t += 215.0
                            spent += 215.0
                        else:
                            break
                return spent

            # AV groups: sub-chunks of <=4 jcs. Chunk i+1's gate opens when
            # chunk i pops (PSUM chain order); a group's first chunk opens
            # when the group two-before finished (2 AV PSUM bufs). GEMM2(g)
            # opens after AV(2,g)'s last chunk + normalize latency.
            av_ord = 0            # append ordinal
            av_group_done = set() # ordinals whose last chunk popped
            av_pending_first = {} # ordinal -> (entry, act_gate)
            av_appended = {}      # g -> number of AV heads appended
            av_done_count = {}    # g -> number of AV heads fully popped
            gemm2_entries = {}

            def append_av_group(h, g):
                nonlocal av_ord
                n = av_ord
                av_ord += 1
                jmax = 4 * g + 3
                parts = [(jlo, min(jlo + 3, jmax)) for jlo in range(0, jmax + 1, 4)]
                entries = []
                for idx, (jlo, jhi) in enumerate(parts):
                    cols = sum(
                        min(512, 512 + 512 * g - 128 * jc)
                        for jc in range(jlo, jhi + 1)
                    )
                    cost = cols / 2.4 + 40.0 * (jhi - jlo + 1)
                    last = jhi == jmax

                    def fn(h=h, g=g, jlo=jlo, jhi=jhi, idx=idx, n=n,
                           cost=cost, last=last):
                        emit_av_part(h, g, jlo, jhi)
                        if idx + 1 < len(entries):
                            # open the next chunk of this group
                            entries[idx + 1][0] = gates[idx + 1]
                        if last:
                            av_group_done.add(n)
                            # release the group two ahead (2 AV PSUM bufs)
                            if n + 2 in av_pending_first:
                                e2, gate2 = av_pending_first.pop(n + 2)
                                e2[0] = gate2
                            av_done_count[g] = av_done_count.get(g, 0) + 1
                            if av_done_count[g] == HPC:
                                # all 3 heads' saT ready soon: open GEMM2
                                for e2 in gemm2_entries[g]:
                                    e2[0] = pe_t + cost + NORM_DELAY

                    entries.append([float("inf"), cost, fn])
                gates = [act_after[(h, jhi)] + SLACK for (jlo, jhi) in parts]
                # first chunk: open if the group two-before is done
                if n < 2 or (n - 2) in av_group_done:
                    entries[0][0] = gates[0]
                else:
                    av_pending_first[n] = (entries[0], gates[0])
                if g not in gemm2_entries:
                    gemm2_entries[g] = [
                        [float("inf"), 620.0,
                         lambda g=g, oc=oc: emit_gemm2_oc(g, oc)]
                        for oc in range(6)
                    ]
                fillers.extend(entries)
                av_appended[g] = av_appended.get(g, 0) + 1
                if av_appended[g] == HPC:
                    fillers.extend(gemm2_entries[g])

            # ---- prefix: enough GEMM1 for the first strips ----
            emit_chain(0, 0)               # kT0 block 0 (g0, h0/h1)
            for isl in range(NISL):
                emit_chain(isl, 1)         # qz0/qz1 complete
            # reset pacing origin at the start of the strip stream
            pe_t = 0.0
            act_t = 0.0

            # hybrid strip order:
            #  group 0 head-blocked, ci2 chains inlined among the h0/h1
            #    strips (all four must precede the first h2 strip: qz2 is
            #    streamed to full N)
            #  groups 1-2 head-interleaved (AV/GEMM2 fillers spread evenly)
            #  group 3 head-blocked (AV(0,3)/AV(1,3) finish early, so the
            #    final AV(2,3) is ~75% done before its last exp; short tail)
            strip_order = []
            for jc in range(4):
                strip_order += [(0, jc)]
            for jc in range(4):
                strip_order += [(1, jc)]
            for jc in range(4):
                strip_order += [(2, jc)]
            for jc in range(4, 12):
                strip_order += [(h, jc) for h in range(HPC)]
            for h in (2, 0, 1):
                strip_order += [(h, jc) for jc in range(12, 16)]

            # inline GEMM1 chains (isl, ci) before their first consumer
            inline_chains = {
                (0, 1): [(0, 2)],   # ci2-isl0
                (0, 3): [(1, 2)],   # ci2-isl1
                (1, 1): [(2, 2)],   # ci2-isl2
                (1, 3): [(3, 2)],   # ci2-isl3  (before first h2 strip)
                (0, 4): [(1, 0)],   # kT0 block for group 1
                (0, 8): [(2, 0)],   # kT0 block for group 2
                (0, 12): [(3, 0)],  # kT0 block for group 3
            }

            for h, jc in strip_order:
                for isl, ci in inline_chains.get((h, jc), []):
                    emit_chain(isl, ci)
                W = N - 128 * jc
                emit_strip(h, jc)
                pe_t += strip_pe_cost(W)
                act_t += strip_act_cost(W)
                act_after[(h, jc)] = act_t
                if jc % 4 == 3:
                    append_av_group(h, jc // 4)
                # keep PE slightly ahead of ACT but not idle
                pop_fillers(act_t - pe_t)

            # drain: keep popping; feed dummies while gates (normalize
            # chains) are still closed, then force-pop in order
            while fillers:
                if pop_fillers(1e9, allow_dummy=False) == 0.0:
                    if n_dummy < 80:
                        emit_dummy()
                        n_dummy += 1
                        pe_t += 215.0
                    else:
                        e = fillers.pop(0)
                        e[2]()
                        pe_t += e[1]

    nc.compile()
    return nc


def _host_prep(x, Wkqv, bkqv, Wproj, bproj):
    f16 = np.float16
    Wk = Wkqv[:, 0:D]
    Wq = Wkqv[:, D : 2 * D]
    Wv = Wkqv[:, 2 * D : 3 * D]
    bk = bkqv[0:D]
    bq = bkqv[D : 2 * D]
    bv = bkqv[2 * D : 3 * D]
    out_bias = (bproj + bv @ Wproj).astype(np.float32)  # softmax rows sum to 1

    ident = np.eye(P, dtype=f16)
    # btri[k, i] = -30000 where k > i: accumulated into scoresT diag blocks,
    # exp((s - 30000) * 0.125) underflows to exactly 0 in fp16.
    btri = (np.tril(np.full((P, P), -30000.0, np.float32), -1)).astype(f16)

    in_maps = []
    for b in range(B):
        xT = x[b].T.astype(f16)                       # [768, 2048]
        # [pi, 3072*isl + 512*dc + c] = xT[128*dc + pi, 512*isl + c]
        xTp = np.ascontiguousarray(
            xT.reshape(6, P, NISL, 512).transpose(1, 2, 0, 3).reshape(P, NISL * 3072)
        )
        for g in range(NG):
            hs = [HPC * g + i for i in range(HPC)]
            wk = [np.asarray(Wk[:, HD * h : HD * h + HD]) for h in hs]
            wq = [np.asarray(Wq[:, HD * h : HD * h + HD]) for h in hs]
            wv = [np.asarray(Wv[:, HD * h : HD * h + HD]) for h in hs]
            # column chunks: ci0 = k01, ci1 = q01, ci2 = k2|q2
            wkq = np.concatenate(
                [wk[0], wk[1], wq[0], wq[1], wk[2], wq[2]], axis=1
            ).astype(np.float32)                       # [768, 384]
            # [pi, 768*ci + 128*dc + c] = wkq[128*dc + pi, 128*ci + c]
            wkqp = np.ascontiguousarray(
                wkq.reshape(6, P, 3, P).transpose(1, 2, 0, 3).reshape(P, 3 * 768)
            ).astype(f16)
            wv_c = np.concatenate(wv, axis=1).astype(np.float32)   # [768, 192]
            # [pi, dc*192 + c] = wv_c[128*dc + pi, c]
            wvp = np.ascontiguousarray(
                wv_c.reshape(6, P, 192).transpose(1, 0, 2).reshape(P, 6 * 192)
            ).astype(f16)
            wp01 = np.concatenate(
                [Wproj[HD * hs[0] : HD * hs[0] + HD, :],
                 Wproj[HD * hs[1] : HD * hs[1] + HD, :]], axis=0
            ).astype(f16)                              # [128, 768]
            wp2 = np.zeros((P, D), f16)                # [128, 768], rows 64+ zero
            wp2[0:64, :] = Wproj[HD * hs[2] : HD * hs[2] + HD, :].astype(f16)
            bkq = np.zeros((P, 4), f16)
            bkq[:, 0] = np.concatenate(
                [bk[HD * hs[0] : HD * hs[0] + HD], bk[HD * hs[1] : HD * hs[1] + HD]]
            )
            bkq[:, 1] = np.concatenate(
                [bq[HD * hs[0] : HD * hs[0] + HD], bq[HD * hs[1] : HD * hs[1] + HD]]
            )
            bkq[0:64, 2] = bk[HD * hs[2] : HD * hs[2] + HD]
            bkq[64:128, 3] = bq[HD * hs[2] : HD * hs[2] + HD]
            # wpackA: bkq | ident | btri | wv;  wpackB: wp01 | wp2
            wpa = np.concatenate([bkq, ident, btri, wvp], axis=1)
            wpb = np.concatenate([wp01, wp2], axis=1)
            in_maps.append(dict(xTp=xTp, wkqp=wkqp, wpa=wpa, wpb=wpb))
    return in_maps, out_bias


def kernel(x, Wkqv, bkqv, Wproj, bproj):
    global _compiled, last_exec_time_ns, last_results
    import concourse.bass_utils as bass_utils

    x = np.asarray(x, np.float32)
    Wkqv = np.asarray(Wkqv, np.float32)
    bkqv = np.asarray(bkqv, np.float32)
    Wproj = np.asarray(Wproj, np.float32)
    bproj = np.asarray(bproj, np.float32)

    if _compiled is None:
        _compiled = _build()
    nc = _compiled

    in_maps, out_bias = _host_prep(x, Wkqv, bkqv, Wproj, bproj)

    trace = os.environ.get("BASS_KERNEL_TRACE", "0") == "1"
    res = bass_utils.run_bass_kernel_spmd(
        nc, in_maps, core_ids=list(range(NCORES)), trace=trace
    )
    last_exec_time_ns = res.exec_time_ns
    last_results = res

    out = np.zeros((B, N, D), np.float32)
    for b in range(B):
        acc = np.zeros((D, N), np.float32)
        for g in range(NG):
            acc += res.results[b * NG + g]["yT"].reshape(D, N).astype(np.float32)
        out[b] = acc.T + out_bias
    return out


# revision 46
# speedup vs baseline: 1.1972x; 1.0699x over previous
"""Causal self-attention (B=2, N=2048, D=768, H=12) on 8 Trainium2 NeuronCores.

Sharding: data-parallel over batch (2) x tensor-parallel over head groups (4),
3 heads per core. Each core computes, for its (batch, head-group):
  GEMM1: kT/qT (transposed) and v (natural) projections from xT,
  scores^T = k @ q^T per head, exp on ScalarE (fp16 out),
  AV with a ones-augmented V giving unnormalized sa + row sums,
  normalize, GEMM2 row-parallel -> yT partial (fp16).
All matmul operands are fp16 (fp32 PSUM accumulate). Host shards inputs, sums
the 4 per-batch partials (the "all-reduce"), and adds the output bias fold
(bproj + bkqv_v @ Wproj - exact because softmax rows sum to 1).

v3 changes vs v2 (trace-driven):
  - exp activations widened to 1024-col PSUM chunks (2-bank sc tiles,
    double-buffered): ~48 fewer ACT instructions, ~10us less ACT busy
  - k/q bias adds moved DVE->ACT (activation Identity with per-partition
    bias AP); they run in phase A where ACT is otherwise idle
  - normalize multiply reads the AV PSUM tile directly (no uav SBUF
    evacuation): ~13us less DVE busy
  - strips emitted head-interleaved; GEMM2 emitted per-oc-chunk
  - filler pop scans past not-yet-ready entries; emits a dummy matmul if
    nothing is ready (a starved PE re-throttles the HAM clock gate)

v4 changes:
  - reciprocal_approx_fast CANNOT read PSUM on hardware (probe: garbage
    results; CoreSim disagrees) - row sum goes through an SBUF copy again
  - ALL matmuls contract over the full 128 partitions: q is stored
    zero-padded per head (qz0=[q0;0], qz1=[0;q1], qz2=[q2;0]), kT1 and
    saT2/wp2 are zero-padded too. Strip/GEMM2 stream cost is unchanged
    (cost = moving cols), but the PE activity monitor appears to weigh
    active rows: K=64-heavy phases ran at K=4/8 clock (1.2GHz) even when
    gap-free, K=128-heavy phases at 2.4GHz. 0*0=0 keeps results exact.
  - GEMM2 readiness keyed off actual AV pop times + normalize latency;
    drain phase keeps the PE fed with dummies while normalize chains run.

v6 changes:
  - exp ACT table preloaded via a tiny dummy activation at kernel start
    (the ~2.7us table load no longer delays the first real exp)
  - GEMM1 fused into the strip stream: only ci0-isl0 + the four ci1
    chains precede the first strip; remaining chains emitted inline just
    before the strips that consume them
  - AV groups split into <=4-matmul sub-chunks with per-chunk readiness
    gates (act watermark of the newest strip the chunk reads). Chunks of
    one group chain-gate each other (PSUM accumulation stays in order);
    groups release two-behind (only 2 AV PSUM bufs). The final AV group
    is now ~75% done before its last exp lands, shrinking the tail.

v7 changes (trace: startup was DMA-trigger-descriptor-generation bound,
~1.3-2.6us of engine time PER dma_start instruction at 128 descriptors):
  - inputs packed into 6 large DMAs (xT in 3 progressive splits, one wkq
    pack, two weight packs), all on the sync ring; ACT issues no DMAs so
    its stream is exps only
  - k/q bias adds stay on ACT (moving them to DVE inflated every exp
    ~190ns - concurrent-engine SBUF contention; measured and reverted)
  - zero-pad memsets moved to GpSimd (frees DVE at startup; library
    reload between Memset and PartitionBroadcast measured at only ~8ns)
  - hybrid strip order: group 0 and 3 head-blocked (early h2 readiness /
    short tail), groups 1-2 head-interleaved (spreads GEMM2 fillers)
  - GEMM2 group-3 evacuations on ACT (idle after the exp stream ends)

Self-contained: hardcodes all shapes; no sibling imports.
"""

import os

import numpy as np

B, N, D = 2, 2048, 768
H, HD = 12, 64
HPC = 3           # heads per core
NG = 4            # head groups
NCORES = 8
P = 128
NJ = N // P       # 16 j-chunks (keys) per head
NISL = 4          # 512-query i-slices

_compiled = None  # cached compiled Bass module
last_exec_time_ns = None
last_results = None

N_WARMUP = 12     # 512-wide dummy matmuls bridging boot -> first GEMM1


def _build():
    import concourse.tile as tile
    import concourse.mybir as mybir
    from concourse import bacc

    f32 = mybir.dt.float32
    f16 = mybir.dt.float16
    ADD = mybir.AluOpType.add
    MULT = mybir.AluOpType.mult
    EXP = mybir.ActivationFunctionType.Exp

    nc = bacc.Bacc(
        "TRN2", target_bir_lowering=False, debug=False, num_devices=NCORES
    )

    # packed DRAM layouts (see _host_prep). Few large DMAs: each dma_start
    # costs the issuing engine ~10-20ns per descriptor (= per partition
    # line), so 13 small loads wedge the ring for ~20us.
    xT_d = nc.dram_tensor("xTp", [P, NISL * 3072], f16, kind="ExternalInput").ap()
    wkq_d = nc.dram_tensor("wkqp", [P, 3 * 768], f16, kind="ExternalInput").ap()
    # wpackA: bkq(4, f16) | ident(128) | btri(128) | wv(1152)
    wpa_d = nc.dram_tensor("wpa", [P, 1412], f16, kind="ExternalInput").ap()
    # wpackB: wp01(768) | wp2(768)
    wpb_d = nc.dram_tensor("wpb", [P, 1536], f16, kind="ExternalInput").ap()
    yT_d = nc.dram_tensor("yT", [6, P, N], f16, kind="ExternalOutput").ap()

    yT_v = yT_d.rearrange("o p f -> p o f")      # [128, 6, 2048]

    with tile.TileContext(nc) as tc:
        import contextlib

        ctx = contextlib.ExitStack()
        with ctx:
            const = ctx.enter_context(tc.tile_pool(name="const", bufs=1))
            big = ctx.enter_context(tc.tile_pool(name="bigbufs", bufs=1))
            work = ctx.enter_context(tc.tile_pool(name="work", bufs=3))
            ypool = ctx.enter_context(tc.tile_pool(name="ypool", bufs=3))
            # PSUM budget (8 banks): sc 2x[128,1024] (4) + gemm 2x[128,512]
            # (2) + av 2x[65,512] (2)
            psum_sc = ctx.enter_context(
                tc.tile_pool(name="psum_sc", bufs=2, space="PSUM")
            )
            psum_gemm = ctx.enter_context(
                tc.tile_pool(name="psum_gemm", bufs=2, space="PSUM")
            )
            psum_av = ctx.enter_context(
                tc.tile_pool(name="psum_av", bufs=2, space="PSUM")
            )

            # ---- SBUF tiles ----
            xT_all = big.tile([P, NISL * 3072], f16, name="xT_all")
            wkq_all = const.tile([P, 3 * 768], f16, name="wkq_all")
            wpa = const.tile([P, 1412], f16, name="wpa")
            wpb = const.tile([P, 1536], f16, name="wpb")
            bkq_t = const.tile([P, 4], f32, name="bkq_t")
            ident_t = wpa[:, 4:132]
            btri_t = wpa[:, 132:260]
            wv_t = wpa[:, 260:1412]
            wp01_t = wpb[:, 0:768]
            wp2_t = wpb[:, 768:1536]
            wkq_ts = [wkq_all[:, 768 * c : 768 * c + 768] for c in range(3)]
            xT_ts = [xT_all[:, 3072 * i : 3072 * i + 3072] for i in range(NISL)]

            # 6 input DMAs, all on the sync ring, priority order: xT-isl0
            # gates the first GEMM1 chain; wpackA carries biases + the mask
            # + wv (early consumers); xT-isl1..3 gate the q projections;
            # wpackB (output projection) is needed last.
            nc.sync.dma_start(xT_ts[0], xT_d[:, 0:3072])
            nc.sync.dma_start(wkq_all[:], wkq_d)
            nc.sync.dma_start(wpa[:], wpa_d)
            nc.sync.dma_start(xT_ts[1], xT_d[:, 3072:6144])
            nc.sync.dma_start(xT_all[:, 6144:12288], xT_d[:, 6144:12288])
            nc.sync.dma_start(wpb[:], wpb_d)
            # biases arrive f16 inside wpackA; DVE wants f32 scalars
            nc.vector.tensor_copy(out=bkq_t[:], in_=wpa[:, 0:4])

            # PE warmup on a zeroed scratch while the first inputs land
            wscr = const.tile([P, 512], f16, name="wscr")
            nc.vector.memset(wscr[:], 0.0)

            # preload the exp ACT table so the ~2.7us load overlaps the
            # input DMA wait instead of delaying the first real exp
            tpre = const.tile([1, 8], f16, name="tpre")
            nc.scalar.activation(tpre[:], wscr[0:1, 0:8], EXP, scale=0.125)

            def emit_dummy():
                wps = psum_gemm.tile([P, 512], f32, tag="ps512", name="wps")
                nc.tensor.matmul(
                    wps[:, 0:512], wscr[:, 0:128], wscr[:], start=True, stop=True
                )

            for _ in range(N_WARMUP):
                emit_dummy()

            # k tiles: kT0 = [k0; k1] on 128 partitions, kT1 = [k2; zeros].
            # q tiles zero-padded per head so strip matmuls contract K=128:
            # qz0 = [q0; 0], qz1 = [0; q1], qz2 = [q2; 0]. The zero halves
            # contribute 0 to the scores; full-K keeps the PE clock warm.
            kT0 = big.tile([P, N], f16, name="kT0")
            kT1 = big.tile([P, N], f16, name="kT1")
            qzs = [big.tile([P, N], f16, name=f"qz{h}") for h in range(HPC)]
            q2st = big.tile([P, N], f16, name="q2st")
            # zero pads on GpSimd (keeps DVE free at startup; the one-off
            # Memset<->PartitionBroadcast library reload measures ~8ns)
            nc.gpsimd.memset(kT1[64:128, :], 0.0)
            nc.gpsimd.memset(qzs[0][64:128, :], 0.0)
            nc.gpsimd.memset(qzs[1][0:64, :], 0.0)
            nc.gpsimd.memset(qzs[2][64:128, :], 0.0)
            vaug = big.tile([P, NJ, HPC, 65], f16, name="vaug")
            nc.vector.memset(vaug[:, :, :, 64:65], 1.0)
            # saT: heads 0+1 packed on 128 partitions; head 2 zero-padded
            saT01s = [big.tile([P, 512], f16, name=f"saT01_{i}") for i in range(4)]
            saT2s = [big.tile([P, 512], f16, name=f"saT2_{i}") for i in range(4)]
            for i in range(4):
                nc.vector.memset(saT2s[i][64:128, :], 0.0)

            # ---- GEMM1 k/q: psum tile per (isl, ci) accumulated over dc ----
            # bias add + cast on ACT (idle during phase A)
            def emit_gemm1_kq(isl, ci):
                ps = psum_gemm.tile([P, 512], f32, tag="ps512", name="ps_kq")
                for dc in range(6):
                    nc.tensor.matmul(
                        ps[:, 0:512],
                        wkq_ts[ci][:, 128 * dc : 128 * dc + 128],
                        xT_ts[isl][:, 512 * dc : 512 * dc + 512],
                        start=(dc == 0),
                        stop=(dc == 5),
                    )
                sl = slice(512 * isl, 512 * isl + 512)
                if ci == 0:
                    nc.scalar.add(kT0[:, sl], ps[:, 0:512], bkq_t[:, 0:1])
                elif ci == 1:
                    nc.scalar.add(
                        qzs[0][0:64, sl], ps[0:64, 0:512], bkq_t[0:64, 1:2]
                    )
                    nc.scalar.add(
                        qzs[1][64:128, sl], ps[64:128, 0:512], bkq_t[64:128, 1:2]
                    )
                else:
                    nc.scalar.add(
                        kT1[0:64, sl], ps[0:64, 0:512], bkq_t[0:64, 2:3]
                    )
                    nc.scalar.add(
                        q2st[64:128, sl], ps[64:128, 0:512], bkq_t[64:128, 3:4]
                    )
                    nc.sync.dma_start(qzs[2][0:64, sl], q2st[64:128, sl])

            # ---- GEMM1 v: one psum tile per 128-query chunk ----
            def emit_gemm1_v(ic):
                ps = psum_gemm.tile([P, 512], f32, tag="ps512", name="ps_v")
                isl, k = divmod(ic, 4)
                for dc in range(6):
                    nc.tensor.matmul(
                        ps[:, 0:192],
                        xT_ts[isl][:, 512 * dc + 128 * k : 512 * dc + 128 * k + 128],
                        wv_t[:, 192 * dc : 192 * dc + 192],
                        start=(dc == 0),
                        stop=(dc == 5),
                    )
                nc.vector.tensor_copy(
                    out=vaug[:, ic, :, 0:64],
                    in_=ps[:, 0:192].rearrange("p (h d) -> p h d", h=HPC),
                )

            # ---- strips: scoresT + causal mask + exp (1024-wide chunks) ----
            all_strips = [[None] * NJ for _ in range(HPC)]

            def emit_strip(h, jc):
                kTc = kT0 if h < 2 else kT1
                qTc = qzs[h]
                i0 = 128 * jc
                W = N - i0
                strip = work.tile(
                    [P, W], f16, tag=f"expT{jc}", bufs=3, name=f"expT{jc}"
                )
                for c0 in range(0, W, 1024):
                    cw = min(1024, W - c0)
                    ps = psum_sc.tile([P, 1024], f32, tag="sc", name="ps_s")
                    for s0 in range(c0, c0 + cw, 512):
                        sw = min(512, W - s0)
                        chained = s0 == 0
                        nc.tensor.matmul(
                            ps[:, s0 - c0 : s0 - c0 + sw],
                            kTc[:, i0 : i0 + 128],
                            qTc[:, i0 + s0 : i0 + s0 + sw],
                            start=True,
                            stop=(not chained),
                        )
                        if chained:
                            # causal mask: accumulate -30000 above the diagonal
                            nc.tensor.matmul(
                                ps[:, 0:128], ident_t[:], btri_t[:],
                                start=False, stop=True,
                            )
                    nc.scalar.activation(
                        strip[:, c0 : c0 + cw], ps[:, 0:cw], EXP, scale=0.125
                    )
                all_strips[h][jc] = strip

            # ---- AV (emitted in sub-chunks) + normalize ----
            av_ps = {}

            def emit_av_part(h, iseg, jlo, jhi):
                strips = all_strips[h]
                jmax = 4 * iseg + 3
                if jlo == 0:
                    av_ps[(h, iseg)] = psum_av.tile(
                        [65, 512], f32, tag="av", name="ps2"
                    )
                ps2 = av_ps[(h, iseg)]
                for jc in range(jlo, jhi + 1):
                    off = 512 * iseg - 128 * jc
                    lo = max(0, off)
                    w = 512 - (lo - off)
                    nc.tensor.matmul(
                        ps2[0:65, 512 - w : 512],
                        vaug[:, jc, h, :],
                        strips[jc][:, lo : lo + w],
                        start=(jc == 0),
                        stop=(jc == jmax),
                    )
                if jhi != jmax:
                    return
                # row sum must bounce through SBUF: reciprocal_approx_fast
                # reads garbage from PSUM on hardware. The multiply below can
                # read PSUM directly. GpSimd runs ONLY partition_broadcast
                # (mixing op types forces library reloads).
                srow = work.tile([1, 512], f32, tag="srow", bufs=2, name="srow")
                nc.vector.tensor_copy(out=srow[:], in_=ps2[64:65, :])
                rrow = work.tile([1, 512], f32, tag="rrow", bufs=2, name="rrow")
                nc.vector.reciprocal_approx_fast(out=rrow[:], in_=srow[:])
                rbc = work.tile([64, 512], f32, tag="rbc", bufs=2, name="rbc")
                nc.gpsimd.partition_broadcast(rbc[:], rrow[:])
                if h == 0:
                    nc.vector.tensor_tensor(
                        saT01s[iseg][0:64, :], ps2[0:64, :], rbc[:], MULT
                    )
                elif h == 1:
                    st1 = work.tile([64, 512], f16, tag="st1", bufs=2, name="st1")
                    nc.vector.tensor_tensor(st1[:], ps2[0:64, :], rbc[:], MULT)
                    nc.sync.dma_start(saT01s[iseg][64:128, :], st1[:])
                else:
                    nc.vector.tensor_tensor(
                        saT2s[iseg][0:64, :], ps2[0:64, :], rbc[:], MULT
                    )

            # ---- GEMM2: both matmuls contract 128 (wp2/saT2 zero-padded) ----
            def emit_gemm2_oc(isl, oc):
                ps = psum_gemm.tile([P, 512], f32, tag="ps512", name="ps_y")
                nc.tensor.matmul(
                    ps[:, 0:512],
                    wp01_t[:, 128 * oc : 128 * oc + 128],
                    saT01s[isl][:],
                    start=True,
                    stop=False,
                )
                nc.tensor.matmul(
                    ps[:, 0:512],
                    wp2_t[:, 128 * oc : 128 * oc + 128],
                    saT2s[isl][:, :],
                    start=False,
                    stop=True,
                )
                yst = ypool.tile([P, 512], f16, tag="yst", name="yst")
                if isl == 3:
                    # group-3 evacuations on ACT: it is idle once the exp
                    # stream has drained, while DVE still runs normalizes
                    nc.scalar.copy(yst[:], ps[:, 0:512])
                else:
                    nc.vector.tensor_copy(out=yst[:], in_=ps[:, 0:512])
                nc.sync.dma_start(
                    yT_v[:, oc, 512 * isl : 512 * isl + 512], yst[:]
                )

            # ---- emission schedule ----
            # Costs in ns for the pacing model (warm clock).
            def strip_pe_cost(W):
                return W / 2.4 + 110 * ((W + 511) // 512) + 160

            def strip_act_cost(W):
                # calibrated: measured exp busy = 0.833ns/col + ~210ns/chunk
                return 0.833 * W + 210 * ((W + 1023) // 1024)

            # fillers: mutable [ready_gate, pe_cost, emit_fn] entries. The
            # gate compares against pe_t (emitted-PE-work watermark).
            fillers = []
            for ic in range(16):
                fillers.append([0.0, 580.0, lambda ic=ic: emit_gemm1_v(ic)])

            pe_t = 0.0    # PE-busy time emitted so far (phase B origin)
            act_t = 0.0   # ACT-busy time emitted so far
            SLACK = 3800.0
            NORM_DELAY = 3000.0   # AV drain -> saT ready (recip+bcast+mult)
            CHAIN = 6 * 512 / 2.4 + 120   # one GEMM1 chain on PE
            n_dummy = 0
            act_after = {}        # (h, jc) -> act_t watermark after its exp

            def emit_chain(isl, ci):
                # GEMM1 chain emitted inline in the strip stream
                nonlocal pe_t, act_t
                emit_gemm1_kq(isl, ci)
                pe_t += CHAIN
                act_t += 700.0 * (1 if ci == 0 else 2)

            def pop_fillers(budget, allow_dummy=True):
                # Pop ready fillers (scanning past not-yet-ready ones; AV
                # chunk/group ordering is enforced via dynamic gates). If
                # nothing is ready and a real deficit remains, emit a dummy
                # matmul: a starved PE re-throttles the HAM clock.
                nonlocal pe_t, n_dummy
                spent = 0.0
                while fillers and spent < budget:
                    for i, e in enumerate(fillers):
                        if e[0] <= pe_t:
                            fillers.pop(i)
                            e[2]()
                            pe_t += e[1]
                            spent += e[1]
                            break
                    else:
                        if allow_dummy and budget - spent > 400.0 and n_dummy < 80:
                            emit_dummy()
                            n_dummy += 1
                            pe_t += 215.0
                            spent += 215.0
                        else:
                            break
                return spent

            # AV groups: sub-chunks of <=4 jcs. Chunk i+1's gate opens when
            # chunk i pops (PSUM chain order); a group's first chunk opens
            # when the group two-before finished (2 AV PSUM bufs). GEMM2(g)
            # opens after AV(2,g)'s last chunk + normalize latency.
            av_ord = 0            # append ordinal
            av_group_done = set() # ordinals whose last chunk popped
            av_pending_first = {} # ordinal -> (entry, act_gate)
            av_appended = {}      # g -> number of AV heads appended
            av_done_count = {}    # g -> number of AV heads fully popped
            gemm2_entries = {}

            def append_av_group(h, g):
                nonlocal av_ord
                n = av_ord
                av_ord += 1
                jmax = 4 * g + 3
                parts = [(jlo, min(jlo + 3, jmax)) for jlo in range(0, jmax + 1, 4)]
                entries = []
                for idx, (jlo, jhi) in enumerate(parts):
                    cols = sum(
                        min(512, 512 + 512 * g - 128 * jc)
                        for jc in range(jlo, jhi + 1)
                    )
                    cost = cols / 2.4 + 40.0 * (jhi - jlo + 1)
                    last = jhi == jmax

                    def fn(h=h, g=g, jlo=jlo, jhi=jhi, idx=idx, n=n,
                           cost=cost, last=last):
                        emit_av_part(h, g, jlo, jhi)
                        if idx + 1 < len(entries):
                            # open the next chunk of this group
                            entries[idx + 1][0] = gates[idx + 1]
                        if last:
                            av_group_done.add(n)
                            # release the group two ahead (2 AV PSUM bufs)
                            if n + 2 in av_pending_first:
                                e2, gate2 = av_pending_first.pop(n + 2)
                                e2[0] = gate2
                            av_done_count[g] = av_done_count.get(g, 0) + 1
                            if av_done_count[g] == HPC:
                                # all 3 heads' saT ready soon: open GEMM2
                                for e2 in gemm2_entries[g]:
                                    e2[0] = pe_t + cost + NORM_DELAY

                    entries.append([float("inf"), cost, fn])
                gates = [act_after[(h, jhi)] + SLACK for (jlo, jhi) in parts]
                # first chunk: open if the group two-before is done
                if n < 2 or (n - 2) in av_group_done:
                    entries[0][0] = gates[0]
                else:
                    av_pending_first[n] = (entries[0], gates[0])
                if g not in gemm2_entries:
                    gemm2_entries[g] = [
                        [float("inf"), 620.0,
                         lambda g=g, oc=oc: emit_gemm2_oc(g, oc)]
                        for oc in range(6)
                    ]
                fillers.extend(entries)
                av_appended[g] = av_appended.get(g, 0) + 1
                if av_appended[g] == HPC:
                    fillers.extend(gemm2_entries[g])

            # ---- prefix: enough GEMM1 for the first strips ----
            emit_chain(0, 0)               # kT0 block 0 (g0, h0/h1)
            for isl in range(NISL):
                emit_chain(isl, 1)         # qz0/qz1 complete
            # reset pacing origin at the start of the strip stream
            pe_t = 0.0
            act_t = 0.0

            # strip order: group-major, head-blocked within each group (the
            # proven v5 backbone). AV(h,g) readiness staggers per head and
            # GEMM2(g) opens at each group's end (spread through the
            # kernel); AV(h,3)'s early sub-chunks still run during earlier
            # groups, keeping the tail short.
            strip_order = []
            for g in range(4):
                for h in range(HPC):
                    strip_order += [(h, jc) for jc in range(4 * g, 4 * g + 4)]

            # inline GEMM1 chains (isl, ci) before their first consumer
            inline_chains = {
                (0, 1): [(0, 2)],   # ci2-isl0
                (0, 3): [(1, 2)],   # ci2-isl1
                (1, 1): [(2, 2)],   # ci2-isl2
                (1, 3): [(3, 2)],   # ci2-isl3  (before first h2 strip)
                (0, 4): [(1, 0)],   # kT0 block for group 1
                (0, 8): [(2, 0)],   # kT0 block for group 2
                (0, 12): [(3, 0)],  # kT0 block for group 3
            }

            for h, jc in strip_order:
                for isl, ci in inline_chains.get((h, jc), []):
                    emit_chain(isl, ci)
                W = N - 128 * jc
                emit_strip(h, jc)
                pe_t += strip_pe_cost(W)
                act_t += strip_act_cost(W)
                act_after[(h, jc)] = act_t
                if jc % 4 == 3:
                    append_av_group(h, jc // 4)
                # keep PE slightly ahead of ACT but not idle
                pop_fillers(act_t - pe_t)

            # drain: keep popping; feed dummies while gates (normalize
            # chains) are still closed, then force-pop in order
            while fillers:
                if pop_fillers(1e9, allow_dummy=False) == 0.0:
                    if n_dummy < 80:
                        emit_dummy()
                        n_dummy += 1
                        pe_t += 215.0
                    else:
                        e = fillers.pop(0)
                        e[2]()
                        pe_t += e[1]

    nc.compile()
    return nc


def _host_prep(x, Wkqv, bkqv, Wproj, bproj):
    f16 = np.float16
    Wk = Wkqv[:, 0:D]
    Wq = Wkqv[:, D : 2 * D]
    Wv = Wkqv[:, 2 * D : 3 * D]
    bk = bkqv[0:D]
    bq = bkqv[D : 2 * D]
    bv = bkqv[2 * D : 3 * D]
    out_bias = (bproj + bv @ Wproj).astype(np.float32)  # softmax rows sum to 1

    ident = np.eye(P, dtype=f16)
    # btri[k, i] = -30000 where k > i: accumulated into scoresT diag blocks,
    # exp((s - 30000) * 0.125) underflows to exactly 0 in fp16.
    btri = (np.tril(np.full((P, P), -30000.0, np.float32), -1)).astype(f16)

    in_maps = []
    for b in range(B):
        xT = x[b].T.astype(f16)                       # [768, 2048]
        # [pi, 3072*isl + 512*dc + c] = xT[128*dc + pi, 512*isl + c]
        xTp = np.ascontiguousarray(
            xT.reshape(6, P, NISL, 512).transpose(1, 2, 0, 3).reshape(P, NISL * 3072)
        )
        for g in range(NG):
            hs = [HPC * g + i for i in range(HPC)]
            wk = [np.asarray(Wk[:, HD * h : HD * h + HD]) for h in hs]
            wq = [np.asarray(Wq[:, HD * h : HD * h + HD]) for h in hs]
            wv = [np.asarray(Wv[:, HD * h : HD * h + HD]) for h in hs]
            # column chunks: ci0 = k01, ci1 = q01, ci2 = k2|q2
            wkq = np.concatenate(
                [wk[0], wk[1], wq[0], wq[1], wk[2], wq[2]], axis=1
            ).astype(np.float32)                       # [768, 384]
            # [pi, 768*ci + 128*dc + c] = wkq[128*dc + pi, 128*ci + c]
            wkqp = np.ascontiguousarray(
                wkq.reshape(6, P, 3, P).transpose(1, 2, 0, 3).reshape(P, 3 * 768)
            ).astype(f16)
            wv_c = np.concatenate(wv, axis=1).astype(np.float32)   # [768, 192]
            # [pi, dc*192 + c] = wv_c[128*dc + pi, c]
            wvp = np.ascontiguousarray(
                wv_c.reshape(6, P, 192).transpose(1, 0, 2).reshape(P, 6 * 192)
            ).astype(f16)
            wp01 = np.concatenate(
                [Wproj[HD * hs[0] : HD * hs[0] + HD, :],
                 Wproj[HD * hs[1] : HD * hs[1] + HD, :]], axis=0
            ).astype(f16)                              # [128, 768]
            wp2 = np.zeros((P, D), f16)                # [128, 768], rows 64+ zero
            wp2[0:64, :] = Wproj[HD * hs[2] : HD * hs[2] + HD, :].astype(f16)
            bkq = np.zeros((P, 4), f16)
            bkq[:, 0] = np.concatenate(
                [bk[HD * hs[0] : HD * hs[0] + HD], bk[HD * hs[1] : HD * hs[1] + HD]]
            )
            bkq[:, 1] = np.concatenate(
                [bq[HD * hs[0] : HD * hs[0] + HD], bq[HD * hs[1] : HD * hs[1] + HD]]
            )
            bkq[0:64, 2] = bk[HD * hs[2] : HD * hs[2] + HD]
            bkq[64:128, 3] = bq[HD * hs[2] : HD * hs[2] + HD]
            # wpackA: bkq | ident | btri | wv;  wpackB: wp01 | wp2
            wpa = np.concatenate([bkq, ident, btri, wvp], axis=1)
            wpb = np.concatenate([wp01, wp2], axis=1)
            in_maps.append(dict(xTp=xTp, wkqp=wkqp, wpa=wpa, wpb=wpb))
    return in_maps, out_bias


def kernel(x, Wkqv, bkqv, Wproj, bproj):
    global _compiled, last_exec_time_ns, last_results
    import concourse.bass_utils as bass_utils

    x = np.asarray(x, np.float32)
    Wkqv = np.asarray(Wkqv, np.float32)
    bkqv = np.asarray(bkqv, np.float32)
    Wproj = np.asarray(Wproj, np.float32)
    bproj = np.asarray(bproj, np.float32)

    if _compiled is None:
        _compiled = _build()
    nc = _compiled

    in_maps, out_bias = _host_prep(x, Wkqv, bkqv, Wproj, bproj)

    trace = os.environ.get("BASS_KERNEL_TRACE", "0") == "1"
    res = bass_utils.run_bass_kernel_spmd(
        nc, in_maps, core_ids=list(range(NCORES)), trace=trace
    )
    last_exec_time_ns = res.exec_time_ns
    last_results = res

    out = np.zeros((B, N, D), np.float32)
    for b in range(B):
        acc = np.zeros((D, N), np.float32)
        for g in range(NG):
            acc += res.results[b * NG + g]["yT"].reshape(D, N).astype(np.float32)
        out[b] = acc.T + out_bias
    return out
